# revision 26
# baseline (speedup 1.0000x reference)
"""VQ codebook lookup (nn_VQ) on 8 TRN2 NeuronCores.

reference: idx = argmin_k ||x_n - e_k||^2 ; out = embeddings[idx]
Equivalent: idx = argmax_k (x_n . e_k - 0.5||e_k||^2)  (||x||^2 is constant per row)

Strategy (data-parallel over N, codebook replicated), transposed-score layout:
  - Host: shard x into 8 x [62500, 100] (pad to [62976, 100] = 123 tiles of
    512 rows), split into bf16 hi/lo, transpose on host. Shipped as TWO
    tensors sized for wide DMA engine spread:
      xA [128, NP]: rows 0:100 x_hi.T, row 100 = 1.0 (bias aug),
                    rows 101:128 = x_lo.T rows 0:27
      xB [73, NP]:  x_lo.T rows 27:100
    Codebook replicated: eth/etl [101, 100] bf16 (e_hi.T / e_lo.T with
    -0.5||e||^2 bias row), m3a/m3b (x_lo-facing slices of e_hi.T, zero-padded
    to legal matmul base partitions) + eg [100, 128] = e_hi padded (gather).
  - Device, per 512-row tile (codebook stationary, x streams; contract over
    d split across xA/xB with PSUM accumulation):
      4 accumulating bf16 matmuls -> scoresT[k, n] f32 PSUM
      ACT copy scoresT -> SBUF; GPSIMD partition_all_reduce(max) -> column
      max replicated on all partitions; DVE is_ge(PSUM scores, max) ->
      one-hot maskT [k, n] bf16 (no transposes anywhere)
      1 matmul eg^T x maskT -> outT[128, n] f32 PSUM (rows 100:128 zero)
      DVE/ACT copy halves -> bf16 out buffer [128, ...]
    DMA in 20-tile groups (~2-2.6 MB HWDGE transfers, contiguous/partition).
  - Host: un-transpose outT rows 0:100 -> [N, 100] f32.
  bf16 hi/lo 3-term scores give ~fp32 argmax accuracy (~3 flips in 500k);
  bf16 output rounds values (total rel err ~5e-3, gate is 2e-2).
"""

import sys

sys.path.insert(0, "/opt/trn_rl_repo")
from contextlib import ExitStack

import ml_dtypes
import numpy as np

import concourse.bass as bass
import concourse.bacc as bacc
import concourse.tile as tile
from concourse import bass_isa, mybir
from concourse._compat import with_exitstack
from concourse.bass_utils import run_bass_kernel_spmd

BF = mybir.dt.bfloat16
F32 = mybir.dt.float32
bf16 = ml_dtypes.bfloat16

N_TOTAL = 500_000
D = 100
K = 100
N_CORES = 8
ST = 512  # rows per tile (one PSUM bank of f32)
N_SHARD = N_TOTAL // N_CORES  # 62500
T = -(-N_SHARD // ST)  # 123 tiles per core
NP = T * ST  # 62976 padded rows per core
G = 20  # tiles per DMA group
NG = -(-T // G)  # 7 groups (6x20 + 3)
DELTA = 4  # software-pipeline depth between score matmuls and gather
SPLIT = 27  # x_lo rows 0:SPLIT live in xA partitions 101:128, rest in xB


@with_exitstack
def _vq_tile_kernel(ctx: ExitStack, tc: tile.TileContext, outT, xA, xB, eth, etl, m3a, m3b, eg):
    nc = tc.nc

    consts = ctx.enter_context(tc.tile_pool(name="consts", bufs=1))
    eth_s = consts.tile([101, K], BF, tag="eth")
    nc.sync.dma_start(eth_s[:], eth[:])
    etl_s = consts.tile([101, K], BF, tag="etl")
    nc.sync.dma_start(etl_s[:], etl[:])
    # m3a weights live at SBUF base partition 64 so lhsT/rhs bases match
    m3a_s = consts.tile([128, K], BF, tag="m3a")
    nc.sync.dma_start(m3a_s[64:128], m3a[:])
    m3b_s = consts.tile([73, K], BF, tag="m3b")
    nc.sync.dma_start(m3b_s[:], m3b[:])
    eg_s = consts.tile([K, 128], BF, tag="eg")
    nc.sync.dma_start(eg_s[:], eg[:])

    xap = ctx.enter_context(tc.tile_pool(name="xa", bufs=2))
    xbp = ctx.enter_context(tc.tile_pool(name="xb", bufs=2))
    obp = ctx.enter_context(tc.tile_pool(name="outbuf", bufs=4))
    scp = ctx.enter_context(tc.tile_pool(name="scoresT", bufs=5, space="PSUM"))
    otp = ctx.enter_context(tc.tile_pool(name="outT", bufs=2, space="PSUM"))
    stp = ctx.enter_context(tc.tile_pool(name="scsb", bufs=3))
    rpp = ctx.enter_context(tc.tile_pool(name="rep", bufs=3))
    mkp = ctx.enter_context(tc.tile_pool(name="maskT", bufs=6))

    xa_v = xA.rearrange("p (t c) -> p t c", c=ST)  # [128, T, 512]
    xb_v = xB.rearrange("p (t c) -> p t c", c=ST)  # [73, T, 512]
    out_v = outT.rearrange("p (t c) -> p t c", c=ST)  # [128, T, 512]

    def gwidth(g):
        return min(G, T - g * G)

    def load_group(g):
        w = gwidth(g)
        xa_t = xap.tile([128, G, ST], BF, tag="xa")
        nc.sync.dma_start(out=xa_t[:, 0:w], in_=xa_v[:, g * G : g * G + w])
        xb_t = xbp.tile([73, G, ST], BF, tag="xb")
        nc.sync.dma_start(out=xb_t[:, 0:w], in_=xb_v[:, g * G : g * G + w])
        return xa_t, xb_t

    cur = load_group(0)
    nxt = None
    ob_cur = None
    mk_ring = {}

    for t in range(T + DELTA):
        if t < T:
            g, i = divmod(t, G)
            if i == 0:
                if g > 0:
                    cur = nxt
                if g + 1 < NG:
                    nxt = load_group(g + 1)
            xa_t, xb_t = cur
            # scoresT[k, n] = x_hi.e_hi + x_hi.e_lo + x_lo.e_hi + bias
            sc = scp.tile([K, ST], F32, tag="sc")
            nc.tensor.matmul(sc[:], eth_s[:], xa_t[0:101, i], start=True, stop=False)
            nc.tensor.matmul(sc[:], etl_s[:], xa_t[0:101, i], start=False, stop=False)
            # m3a rows 0:37 are zero (they face hi/aug partitions 64:101);
            # rows 37:64 = e_hi.T[0:27] facing x_lo rows 0:27 at partitions 101:128
            nc.tensor.matmul(sc[:], m3a_s[64:128], xa_t[64:128, i], start=False, stop=False)
            nc.tensor.matmul(sc[:], m3b_s[:], xb_t[:, i], start=False, stop=True)
            # column max replicated to all partitions (gpsimd reads SBUF only)
            st = stp.tile([K, ST], F32, tag="st")
            nc.scalar.copy(st[:], sc[:])
            rp = rpp.tile([K, ST], F32, tag="rp")
            nc.gpsimd.partition_all_reduce(rp[:], st[:], K, bass_isa.ReduceOp.max)
            mk = mkp.tile([K, ST], BF, tag="mk")
            # in0 from PSUM: a 2-port SBUF read here starves gpsimd (SBUF port lock)
            nc.vector.tensor_tensor(out=mk[:], in0=sc[:], in1=rp[:], op=mybir.AluOpType.is_ge)
            mk_ring[t] = mk
        if t >= DELTA:
            s = t - DELTA
            ot = otp.tile([128, ST], F32, tag="ot")
            nc.tensor.matmul(ot[:], eg_s[:], mk_ring.pop(s)[:], start=True, stop=True)
            half = ST // 2
            ob = obp.tile([128, ST], BF, tag="ob")
            nc.vector.tensor_scalar_add(ob[:, 0:half], ot[:, 0:half], 0.0)
            nc.scalar.copy(ob[:, half:ST], ot[:, half:ST])
            # Per-tile store on the sync ring with small (512B) descriptors —
            # mimics the one store shape observed to spray across all 16 SDMA
            # engines in kernel context (group-sized contiguous stores pin
            # ~60% of their bytes on one ~27 GB/s engine).
            nc.sync.dma_start(out=out_v[:, s], in_=ob[:], max_dma_last_dim=256)


def build_nc():
    nc = bacc.Bacc(
        "TRN2",
        target_bir_lowering=False,
        debug=False,
        enable_asserts=True,
        num_devices=N_CORES,
    )
    outT = nc.dram_tensor("outT", [128, NP], BF, kind="ExternalOutput").ap()
    xA = nc.dram_tensor("xA", [128, NP], BF, kind="ExternalInput").ap()
    xB = nc.dram_tensor("xB", [73, NP], BF, kind="ExternalInput").ap()
    eth = nc.dram_tensor("eth", [101, K], BF, kind="ExternalInput").ap()
    etl = nc.dram_tensor("etl", [101, K], BF, kind="ExternalInput").ap()
    m3a = nc.dram_tensor("m3a", [64, K], BF, kind="ExternalInput").ap()
    m3b = nc.dram_tensor("m3b", [73, K], BF, kind="ExternalInput").ap()
    eg = nc.dram_tensor("eg", [K, 128], BF, kind="ExternalInput").ap()
    with tile.TileContext(nc) as tc:
        _vq_tile_kernel(tc, outT, xA, xB, eth, etl, m3a, m3b, eg)
    nc.compile()
    return nc


def prep_inputs(inputs: np.ndarray, embeddings: np.ndarray):
    """Host-side shard + layout prep. Returns in_maps for the 8 cores."""
    x = np.ascontiguousarray(inputs, dtype=np.float32)
    e = np.ascontiguousarray(embeddings, dtype=np.float32)

    e_hi = e.astype(bf16)
    e_lo = (e - e_hi.astype(np.float32)).astype(bf16)
    bias = (-0.5 * np.sum(e.astype(np.float64) ** 2, axis=1)).astype(np.float32)
    b_hi = bias.astype(bf16)
    b_lo = (bias - b_hi.astype(np.float32)).astype(bf16)
    eth = np.zeros((101, K), dtype=bf16)
    eth[0:D] = e_hi.T
    eth[100] = b_hi
    etl = np.zeros((101, K), dtype=bf16)
    etl[0:D] = e_lo.T
    etl[100] = b_lo
    m3a = np.zeros((64, K), dtype=bf16)
    m3a[64 - SPLIT : 64] = e_hi.T[0:SPLIT]
    m3b = np.zeros((73, K), dtype=bf16)
    m3b[:] = e_hi.T[SPLIT:D]
    eg = np.zeros((K, 128), dtype=bf16)
    eg[:, 0:D] = e_hi

    xT = np.ascontiguousarray(x.T)  # [100, N] f32
    xh = xT.astype(bf16)
    xl = (xT - xh.astype(np.float32)).astype(bf16)

    in_maps = []
    for i in range(N_CORES):
        lo_c, hi_c = i * N_SHARD, (i + 1) * N_SHARD
        xA = np.zeros((128, NP), dtype=bf16)
        xA[0:D, 0:N_SHARD] = xh[:, lo_c:hi_c]
        xA[100, :] = 1.0
        xA[101:128, 0:N_SHARD] = xl[0:SPLIT, lo_c:hi_c]
        xB = np.zeros((73, NP), dtype=bf16)
        xB[:, 0:N_SHARD] = xl[SPLIT:D, lo_c:hi_c]
        in_maps.append(
            {"xA": xA, "xB": xB, "eth": eth, "etl": etl, "m3a": m3a, "m3b": m3b, "eg": eg}
        )
    return in_maps


_NC_CACHE = None


def kernel(inputs: np.ndarray, embeddings: np.ndarray) -> np.ndarray:
    global _NC_CACHE
    if _NC_CACHE is None:
        _NC_CACHE = build_nc()
    nc = _NC_CACHE
    in_maps = prep_inputs(inputs, embeddings)
    res = run_bass_kernel_spmd(nc, in_maps, core_ids=list(range(N_CORES)))
    out = np.empty((N_TOTAL, D), dtype=np.float32)
    for i in range(N_CORES):
        shard = res.results[i]["outT"][0:D, 0:N_SHARD]  # [100, 62500] bf16
        out[i * N_SHARD : (i + 1) * N_SHARD] = shard.astype(np.float32).T
    return out


# revision 28
# speedup vs baseline: 1.1599x; 1.1599x over previous
"""VQ codebook lookup (nn_VQ) on 8 TRN2 NeuronCores.

reference: idx = argmin_k ||x_n - e_k||^2 ; out = embeddings[idx]
Equivalent: idx = argmax_k (x_n . e_k - 0.5||e_k||^2)  (||x||^2 is constant per row)

Strategy (data-parallel over N, codebook replicated), transposed-score layout:
  - Host: shard x into 8 x [62500, 100] (pad to [62976, 100] = 123 tiles of
    512 rows), split into bf16 hi/lo, transpose on host. Shipped as TWO
    tensors sized for wide DMA engine spread:
      xA [128, NP]: rows 0:100 x_hi.T, row 100 = 1.0 (bias aug),
                    rows 101:128 = x_lo.T rows 0:27
      xB [73, NP]:  x_lo.T rows 27:100
    Codebook replicated: eth/etl [101, 100] bf16 (e_hi.T / e_lo.T with
    -0.5||e||^2 bias row), m3a/m3b (x_lo-facing slices of e_hi.T, zero-padded
    to legal matmul base partitions) + eg [100, 128] = e_hi padded (gather).
  - Device, per 512-row tile (codebook stationary, x streams; contract over
    d split across xA/xB with PSUM accumulation):
      4 accumulating bf16 matmuls -> scoresT[k, n] f32 PSUM
      ACT copy scoresT -> SBUF; GPSIMD partition_all_reduce(max) -> column
      max replicated on all partitions; DVE is_ge(PSUM scores, max) ->
      one-hot maskT [k, n] bf16 (no transposes anywhere)
      1 matmul eg^T x maskT -> outT[128, n] f32 PSUM (rows 100:128 zero)
      DVE/ACT copy halves -> bf16 out buffer [128, ...]
    DMA in 20-tile groups (~2-2.6 MB HWDGE transfers, contiguous/partition).
  - Host: un-transpose outT rows 0:100 -> [N, 100] f32.
  bf16 hi/lo 3-term scores give ~fp32 argmax accuracy (~3 flips in 500k);
  bf16 output rounds values (total rel err ~5e-3, gate is 2e-2).
"""

import sys

sys.path.insert(0, "/opt/trn_rl_repo")
from contextlib import ExitStack

import ml_dtypes
import numpy as np

import concourse.bass as bass
import concourse.bacc as bacc
import concourse.tile as tile
from concourse import bass_isa, mybir
from concourse._compat import with_exitstack
from concourse.bass_utils import run_bass_kernel_spmd

BF = mybir.dt.bfloat16
F32 = mybir.dt.float32
bf16 = ml_dtypes.bfloat16

N_TOTAL = 500_000
D = 100
K = 100
N_CORES = 8
ST = 512  # rows per tile (one PSUM bank of f32)
N_SHARD = N_TOTAL // N_CORES  # 62500
T = -(-N_SHARD // ST)  # 123 tiles per core
NP = T * ST  # 62976 padded rows per core
G = 20  # tiles per DMA group
NG = -(-T // G)  # 7 groups (6x20 + 3)
DELTA = 4  # software-pipeline depth between score matmuls and gather
SPLIT = 27  # x_lo rows 0:SPLIT live in xA partitions 101:128, rest in xB


@with_exitstack
def _vq_tile_kernel(ctx: ExitStack, tc: tile.TileContext, outT, xA, xB, eth, etl, m3a, m3b, eg):
    nc = tc.nc

    consts = ctx.enter_context(tc.tile_pool(name="consts", bufs=1))
    eth_s = consts.tile([101, K], BF, tag="eth")
    nc.sync.dma_start(eth_s[:], eth[:])
    etl_s = consts.tile([101, K], BF, tag="etl")
    nc.sync.dma_start(etl_s[:], etl[:])
    # m3a weights live at SBUF base partition 64 so lhsT/rhs bases match
    m3a_s = consts.tile([128, K], BF, tag="m3a")
    nc.sync.dma_start(m3a_s[64:128], m3a[:])
    m3b_s = consts.tile([73, K], BF, tag="m3b")
    nc.sync.dma_start(m3b_s[:], m3b[:])
    eg_s = consts.tile([K, 128], BF, tag="eg")
    nc.sync.dma_start(eg_s[:], eg[:])

    xap = ctx.enter_context(tc.tile_pool(name="xa", bufs=2))
    xbp = ctx.enter_context(tc.tile_pool(name="xb", bufs=2))
    obp = ctx.enter_context(tc.tile_pool(name="outbuf", bufs=2))
    scp = ctx.enter_context(tc.tile_pool(name="scoresT", bufs=5, space="PSUM"))
    otp = ctx.enter_context(tc.tile_pool(name="outT", bufs=2, space="PSUM"))
    stp = ctx.enter_context(tc.tile_pool(name="scsb", bufs=3))
    rpp = ctx.enter_context(tc.tile_pool(name="rep", bufs=3))
    mkp = ctx.enter_context(tc.tile_pool(name="maskT", bufs=6))

    xa_v = xA.rearrange("p (t c) -> p t c", c=ST)  # [128, T, 512]
    xb_v = xB.rearrange("p (t c) -> p t c", c=ST)  # [73, T, 512]
    out_v = outT.rearrange("p (t c) -> p t c", c=ST)  # [128, T, 512]

    def gwidth(g):
        return min(G, T - g * G)

    def load_group(g):
        w = gwidth(g)
        xa_t = xap.tile([128, G, ST], BF, tag="xa")
        nc.sync.dma_start(out=xa_t[:, 0:w], in_=xa_v[:, g * G : g * G + w])
        xb_t = xbp.tile([73, G, ST], BF, tag="xb")
        nc.sync.dma_start(out=xb_t[:, 0:w], in_=xb_v[:, g * G : g * G + w])
        return xa_t, xb_t

    cur = load_group(0)
    nxt = None
    ob_cur = None
    mk_ring = {}

    for t in range(T + DELTA):
        if t < T:
            g, i = divmod(t, G)
            if i == 0:
                if g > 0:
                    cur = nxt
                if g + 1 < NG:
                    nxt = load_group(g + 1)
            xa_t, xb_t = cur
            # scoresT[k, n] = x_hi.e_hi + x_hi.e_lo + x_lo.e_hi + bias
            sc = scp.tile([K, ST], F32, tag="sc")
            nc.tensor.matmul(sc[:], eth_s[:], xa_t[0:101, i], start=True, stop=False)
            nc.tensor.matmul(sc[:], etl_s[:], xa_t[0:101, i], start=False, stop=False)
            # m3a rows 0:37 are zero (they face hi/aug partitions 64:101);
            # rows 37:64 = e_hi.T[0:27] facing x_lo rows 0:27 at partitions 101:128
            nc.tensor.matmul(sc[:], m3a_s[64:128], xa_t[64:128, i], start=False, stop=False)
            nc.tensor.matmul(sc[:], m3b_s[:], xb_t[:, i], start=False, stop=True)
            # column max replicated to all partitions (gpsimd reads SBUF only)
            st = stp.tile([K, ST], F32, tag="st")
            nc.scalar.copy(st[:], sc[:])
            rp = rpp.tile([K, ST], F32, tag="rp")
            nc.gpsimd.partition_all_reduce(rp[:], st[:], K, bass_isa.ReduceOp.max)
            mk = mkp.tile([K, ST], BF, tag="mk")
            # in0 from PSUM: a 2-port SBUF read here starves gpsimd (SBUF port lock)
            nc.vector.tensor_tensor(out=mk[:], in0=sc[:], in1=rp[:], op=mybir.AluOpType.is_ge)
            mk_ring[t] = mk
        if t >= DELTA:
            s = t - DELTA
            gs, si = divmod(s, G)
            if si == 0:
                ob_cur = obp.tile([128, G, ST], BF, tag="ob")
            ot = otp.tile([128, ST], F32, tag="ot")
            nc.tensor.matmul(ot[:], eg_s[:], mk_ring.pop(s)[:], start=True, stop=True)
            half = ST // 2
            nc.vector.tensor_scalar_add(ob_cur[:, si, 0:half], ot[:, 0:half], 0.0)
            nc.scalar.copy(ob_cur[:, si, half:ST], ot[:, half:ST])
            w = gwidth(gs)
            if si == w - 1:
                nc.scalar.dma_start(
                    out=out_v[:, gs * G : gs * G + w],
                    in_=ob_cur[:, 0:w],
                    max_dma_last_dim=512,
                )


def build_nc():
    nc = bacc.Bacc(
        "TRN2",
        target_bir_lowering=False,
        debug=False,
        enable_asserts=True,
        num_devices=N_CORES,
    )
    outT = nc.dram_tensor("outT", [128, NP], BF, kind="ExternalOutput").ap()
    xA = nc.dram_tensor("xA", [128, NP], BF, kind="ExternalInput").ap()
    xB = nc.dram_tensor("xB", [73, NP], BF, kind="ExternalInput").ap()
    eth = nc.dram_tensor("eth", [101, K], BF, kind="ExternalInput").ap()
    etl = nc.dram_tensor("etl", [101, K], BF, kind="ExternalInput").ap()
    m3a = nc.dram_tensor("m3a", [64, K], BF, kind="ExternalInput").ap()
    m3b = nc.dram_tensor("m3b", [73, K], BF, kind="ExternalInput").ap()
    eg = nc.dram_tensor("eg", [K, 128], BF, kind="ExternalInput").ap()
    with tile.TileContext(nc) as tc:
        _vq_tile_kernel(tc, outT, xA, xB, eth, etl, m3a, m3b, eg)
    nc.compile()
    return nc


def prep_inputs(inputs: np.ndarray, embeddings: np.ndarray):
    """Host-side shard + layout prep. Returns in_maps for the 8 cores."""
    x = np.ascontiguousarray(inputs, dtype=np.float32)
    e = np.ascontiguousarray(embeddings, dtype=np.float32)

    e_hi = e.astype(bf16)
    e_lo = (e - e_hi.astype(np.float32)).astype(bf16)
    bias = (-0.5 * np.sum(e.astype(np.float64) ** 2, axis=1)).astype(np.float32)
    b_hi = bias.astype(bf16)
    b_lo = (bias - b_hi.astype(np.float32)).astype(bf16)
    eth = np.zeros((101, K), dtype=bf16)
    eth[0:D] = e_hi.T
    eth[100] = b_hi
    etl = np.zeros((101, K), dtype=bf16)
    etl[0:D] = e_lo.T
    etl[100] = b_lo
    m3a = np.zeros((64, K), dtype=bf16)
    m3a[64 - SPLIT : 64] = e_hi.T[0:SPLIT]
    m3b = np.zeros((73, K), dtype=bf16)
    m3b[:] = e_hi.T[SPLIT:D]
    eg = np.zeros((K, 128), dtype=bf16)
    eg[:, 0:D] = e_hi

    xT = np.ascontiguousarray(x.T)  # [100, N] f32
    xh = xT.astype(bf16)
    xl = (xT - xh.astype(np.float32)).astype(bf16)

    in_maps = []
    for i in range(N_CORES):
        lo_c, hi_c = i * N_SHARD, (i + 1) * N_SHARD
        xA = np.zeros((128, NP), dtype=bf16)
        xA[0:D, 0:N_SHARD] = xh[:, lo_c:hi_c]
        xA[100, :] = 1.0
        xA[101:128, 0:N_SHARD] = xl[0:SPLIT, lo_c:hi_c]
        xB = np.zeros((73, NP), dtype=bf16)
        xB[:, 0:N_SHARD] = xl[SPLIT:D, lo_c:hi_c]
        in_maps.append(
            {"xA": xA, "xB": xB, "eth": eth, "etl": etl, "m3a": m3a, "m3b": m3b, "eg": eg}
        )
    return in_maps


_NC_CACHE = None


def kernel(inputs: np.ndarray, embeddings: np.ndarray) -> np.ndarray:
    global _NC_CACHE
    if _NC_CACHE is None:
        _NC_CACHE = build_nc()
    nc = _NC_CACHE
    in_maps = prep_inputs(inputs, embeddings)
    res = run_bass_kernel_spmd(nc, in_maps, core_ids=list(range(N_CORES)))
    out = np.empty((N_TOTAL, D), dtype=np.float32)
    for i in range(N_CORES):
        shard = res.results[i]["outT"][0:D, 0:N_SHARD]  # [100, 62500] bf16
        out[i * N_SHARD : (i + 1) * N_SHARD] = shard.astype(np.float32).T
    return out
